# revision 26
# baseline (speedup 1.0000x reference)
"""HGAT model kernel for 8x Trainium2 NeuronCores.

Structure: 2-layer GRU (T=60, H=128) data-parallel over N=8192 nodes
(1024/core), then hypergraph attention with the [N,N] matrix algebraically
collapsed through the E=30 hyperedge dimension (AllReduces of [30,1] and
[30,128]).

GRU engine assignment (per layer-step, [128,1024] ops):
  PE:   whh/wih matmuls (bf16) + identity-matmul accumulation of r*(hn+b)
        into the n-gate PSUM (replaces a DVE add)
  Act:  sigmoid(r), sigmoid(w) [w = 1-z via negated z weights/biases],
        tanh(n)  -- the only engine with transcendentals
  DVE:  rh = r*(hn+b) (stt), d = n-h, e = w*d, h' = h+e
  Pool: nothing (gpsimd tensor ops run at 0.42 efficiency and have no
        PSUM port -- the old baseline bottleneck)
L1 lags L0 by one timestep so the two recurrent chains pipeline.
"""

import sys

sys.path.insert(0, "/opt/trn_rl_repo")

import os
import numpy as np
import ml_dtypes

import concourse.bacc as bacc
import concourse.tile as tile
import concourse.mybir as mybir
from concourse.bass_utils import run_bass_kernel_spmd

F32 = mybir.dt.float32
BF16 = mybir.dt.bfloat16
AF = mybir.ActivationFunctionType
ALU = mybir.AluOpType
AX = mybir.AxisListType

N = 8192
T = int(os.environ.get("KERNEL_T", "60"))
DF = 6
H = 128
E = 30
NC = 8
NL = N // NC          # 1024 nodes per core
NCH = NL // 128       # 8 chunks of 128 nodes
SLOPE = 0.01
DEBUG = bool(int(os.environ.get("KERNEL_DEBUG", "0")))

_CACHE = {}


def _build_program():
    nc = bacc.Bacc("TRN2", target_bir_lowering=False, debug=False, num_devices=NC)

    dt = BF16

    # ---- DRAM I/O ----
    x_d = nc.dram_tensor("x", [7, T * NL], dt, kind="ExternalInput")
    whhT0_d = nc.dram_tensor("whhT0", [H, 3 * H], dt, kind="ExternalInput")
    wihT0_d = nc.dram_tensor("wihT0", [7, 3 * H], dt, kind="ExternalInput")
    whhT1_d = nc.dram_tensor("whhT1", [H, 3 * H], dt, kind="ExternalInput")
    wihT1_d = nc.dram_tensor("wihT1", [H, 3 * H], dt, kind="ExternalInput")
    bias_d = nc.dram_tensor("bias", [H, 8], F32, kind="ExternalInput")
    v1_d = nc.dram_tensor("v1", [H, 1], dt, kind="ExternalInput")
    v2_d = nc.dram_tensor("v2", [H, 1], F32, kind="ExternalInput")
    wfcT_d = nc.dram_tensor("wfcT", [H, H], F32, kind="ExternalInput")
    wout_d = nc.dram_tensor("wout", [H, 1], F32, kind="ExternalInput")
    identd_d = nc.dram_tensor("identd", [H, H], dt, kind="ExternalInput")
    identf_d = nc.dram_tensor("identf", [H, H], F32, kind="ExternalInput")
    gh_d = nc.dram_tensor("gh", [128, NCH * E], F32, kind="ExternalInput")
    invdv_d = nc.dram_tensor("invdv", [128, NCH], F32, kind="ExternalInput")
    invde_d = nc.dram_tensor("invde", [E, 1], F32, kind="ExternalInput")
    ones_d = nc.dram_tensor("ones1", [1, H], F32, kind="ExternalInput")
    y_d = nc.dram_tensor("y", [1, NL], F32, kind="ExternalOutput")
    if DEBUG:
        dbg_h0_d = nc.dram_tensor("dbg_h0", [H, NL], F32, kind="ExternalOutput")
        dbg_h1_d = nc.dram_tensor("dbg_h1", [H, NL], F32, kind="ExternalOutput")
        dbg_s1_d = nc.dram_tensor("dbg_s1", [128, NCH], F32, kind="ExternalOutput")
        dbg_s2_d = nc.dram_tensor("dbg_s2", [E, 1], F32, kind="ExternalOutput")
        dbg_att_d = nc.dram_tensor("dbg_att", [128, NCH * E], F32, kind="ExternalOutput")
        dbg_S_d = nc.dram_tensor("dbg_S", [E, H], F32, kind="ExternalOutput")

    with tile.TileContext(nc) as tc:
        with (
            tc.tile_pool(name="const", bufs=1) as cp,
            tc.tile_pool(name="xp", bufs=1) as xp,
            tc.tile_pool(name="hp", bufs=3) as hp,
            tc.tile_pool(name="wk", bufs=3) as wk,
            tc.tile_pool(name="pbs", bufs=1) as pbs,
            tc.tile_pool(name="dram", bufs=1, space="DRAM") as dp,
        ):
            # ---- load constants ----
            def cload(dram, shape, dtype):
                t_ = cp.tile(shape, dtype, tag=dram.name)
                nc.sync.dma_start(t_[:], dram[:])
                return t_

            # first-round dependencies first: the startup DMA issues are
            # serialized (~0.7us each), so order by when the GRU needs them
            whhT0 = cload(whhT0_d, [H, 3 * H], dt)
            wihT0 = cload(wihT0_d, [7, 3 * H], dt)
            x_sb = xp.tile([7, T * NL], dt, tag="x_sb")
            NQ = 8
            for q in range(2):
                s = slice(q * T * NL // NQ, (q + 1) * T * NL // NQ)
                nc.sync.dma_start(x_sb[:, s], x_d[:, s])
            whhT1 = cload(whhT1_d, [H, 3 * H], dt)
            wihT1 = cload(wihT1_d, [H, 3 * H], dt)
            bias = cload(bias_d, [H, 8], F32)
            identd = cload(identd_d, [H, H], dt)
            for q in range(2, NQ):
                s = slice(q * T * NL // NQ, (q + 1) * T * NL // NQ)
                nc.sync.dma_start(x_sb[:, s], x_d[:, s])
            v1 = cload(v1_d, [H, 1], dt)
            v2 = cload(v2_d, [H, 1], F32)
            wfcT = cload(wfcT_d, [H, H], F32)
            wout = cload(wout_d, [H, 1], F32)
            identf = cload(identf_d, [H, H], F32)
            gh = cload(gh_d, [128, NCH * E], F32)
            invdv = cload(invdv_d, [128, NCH], F32)
            invde = cload(invde_d, [E, 1], F32)
            ones1 = cload(ones_d, [1, H], F32)

            h0 = hp.tile([H, NL], dt, tag="h0")
            h1 = hp.tile([H, NL], dt, tag="h1")
            nc.vector.memzero(h0[:])
            nc.vector.memzero(h1[:])

            # slices of the 3H weight dim
            RS = slice(0, H)
            WS = slice(H, 2 * H)
            NS = slice(2 * H, 3 * H)

            with tc.tile_pool(name="psA", bufs=1, space="PSUM") as psA:

                def gru_gates(whh, wih, h_prev, xin, b_r, b_w, b_hn, b_in):
                    """One GRU layer-step through tanh; returns (nn, wg, pp).
                    whh/wih: [*, 3H] stationaries (z block negated).
                    b_r/b_w/b_in: activation bias APs or None (L0 folds them
                    into the x ones-row). b_hn: stt scalar AP (bhh_n).
                    PSUM tags 'g' (r then w) and 'hi' (hn then in) rotate
                    with bufs=2 so the two layers' chains decouple."""
                    ps_r = psA.tile([H, NL], F32, tag="ps_g", bufs=2)
                    for c in (0, 512):
                        nc.tensor.matmul(ps_r[:, c:c+512], whh[:, RS], h_prev[:, c:c+512], start=True, stop=False)
                    for c in (0, 512):
                        nc.tensor.matmul(ps_r[:, c:c+512], wih[:, RS], xin[:, c:c+512], start=False, stop=True)
                    rg = wk.tile([H, NL], dt, tag="rg")
                    if b_r is None:
                        nc.scalar.activation(rg[:], ps_r[:], AF.Sigmoid)
                    else:
                        nc.scalar.activation(rg[:], ps_r[:], AF.Sigmoid, bias=b_r)
                    ps_hn = psA.tile([H, NL], F32, tag="ps_hi", bufs=2)
                    for c in (0, 512):
                        nc.tensor.matmul(ps_hn[:, c:c+512], whh[:, NS], h_prev[:, c:c+512], start=True, stop=True)
                    rh = wk.tile([H, NL], dt, tag="rh")
                    nc.vector.scalar_tensor_tensor(
                        rh[:], ps_hn[:], b_hn, rg[:], ALU.add, ALU.mult,
                    )
                    # w-gate psum reuses the 'g' slot pair
                    ps_w = psA.tile([H, NL], F32, tag="ps_g", bufs=2)
                    for c in (0, 512):
                        nc.tensor.matmul(ps_w[:, c:c+512], whh[:, WS], h_prev[:, c:c+512], start=True, stop=False)
                    for c in (0, 512):
                        nc.tensor.matmul(ps_w[:, c:c+512], wih[:, WS], xin[:, c:c+512], start=False, stop=True)
                    wg = wk.tile([H, NL], dt, tag="wg")
                    if b_w is None:
                        nc.scalar.activation(wg[:], ps_w[:], AF.Sigmoid)
                    else:
                        nc.scalar.activation(wg[:], ps_w[:], AF.Sigmoid, bias=b_w)
                    # n-gate: i_n matmul + identity-accumulated rh + tanh
                    ps_in = psA.tile([H, NL], F32, tag="ps_hi", bufs=2)
                    for c in (0, 512):
                        nc.tensor.matmul(ps_in[:, c:c+512], wih[:, NS], xin[:, c:c+512], start=True, stop=False)
                    for c in (0, 512):
                        nc.tensor.matmul(ps_in[:, c:c+512], identd[:], rh[:, c:c+512], start=False, stop=True)
                    nn = wk.tile([H, NL], dt, tag="nn")
                    if b_in is None:
                        nc.scalar.activation(nn[:], ps_in[:], AF.Tanh)
                    else:
                        nc.scalar.activation(nn[:], ps_in[:], AF.Tanh, bias=b_in)
                    return nn, wg, h_prev

                def gru_update(nn, wg, h_prev, htag):
                    # h' = h + w*(n-h)
                    dd = wk.tile([H, NL], dt, tag="dd")
                    nc.vector.tensor_sub(dd[:], nn[:], h_prev[:])
                    ee = wk.tile([H, NL], dt, tag="ee")
                    nc.vector.tensor_mul(ee[:], wg[:], dd[:])
                    h_new = hp.tile([H, NL], dt, tag=htag)
                    nc.vector.tensor_add(h_new[:], h_prev[:], ee[:])
                    return h_new

                # Emission per round: both layers' gate phases first, then
                # both updates -- keeps each in-order engine queue from
                # head-of-line blocking on the other layer's chain.
                h0_hist = [h0]  # h0 after step t, for the lagged L1
                for k in range(T + 1):
                    a0 = a1 = None
                    if k < T:
                        xt = x_sb[:, NL * k : NL * (k + 1)]
                        a0 = gru_gates(
                            whhT0, wihT0, h0, xt,
                            None, None, bias[:, 2:3], None,
                        )
                    if k >= 1:
                        # L1 lags one step: computes step k-1 with input
                        # ys[k-1] = h0_hist[k] (created last round)
                        a1 = gru_gates(
                            whhT1, wihT1, h1, h0_hist[k],
                            bias[:, 0:1], bias[:, 1:2], bias[:, 3:4],
                            bias[:, 4:5],
                        )
                    if a0 is not None:
                        h0 = gru_update(a0[0], a0[1], a0[2], "h0")
                        h0_hist.append(h0)
                    if a1 is not None:
                        h1 = gru_update(a1[0], a1[1], a1[2], "h1")

            if DEBUG:
                dbg_h0 = pbs.tile([H, NL], F32, tag="dbg_h0")
                nc.vector.tensor_copy(dbg_h0[:], h0[:])
                nc.sync.dma_start(dbg_h0_d[:], dbg_h0[:])
                dbg_h1 = pbs.tile([H, NL], F32, tag="dbg_h1")
                nc.vector.tensor_copy(dbg_h1[:], h1[:])
                nc.sync.dma_start(dbg_h1_d[:], dbg_h1[:])

            # ---- attention head ----
            hid_nm = pbs.tile([128, NL], F32)  # node-major hidden
            s1 = pbs.tile([128, NCH], F32)
            aggT = pbs.tile([H, E], F32)
            s2p = pbs.tile([E, 1], F32)
            with tc.tile_pool(name="psB1", bufs=1, space="PSUM") as pb1:
                ps_agg = pb1.tile([H, E], F32, tag="agg")
                ps_s1 = pb1.tile([128, NCH], F32, tag="s1")
                for c in range(NCH):
                    cs = slice(128 * c, 128 * (c + 1))
                    ps_tr = pb1.tile([128, 128], dt, tag="tr", bufs=2)
                    nc.tensor.transpose(ps_tr[:], h1[:, cs], identd[:])
                    nc.scalar.copy(hid_nm[:, cs], ps_tr[:])
                    nc.tensor.matmul(
                        ps_agg[:], hid_nm[:, cs], gh[:, E * c : E * (c + 1)],
                        start=(c == 0), stop=(c == NCH - 1),
                    )
                    nc.tensor.matmul(
                        ps_s1[:, c : c + 1], h1[:, cs], v1[:],
                        start=True, stop=True,
                    )
                nc.scalar.copy(aggT[:], ps_agg[:])
                nc.scalar.copy(s1[:], ps_s1[:])
                # partial edge score: s2_part = aggT^T @ v2  -> [E,1]
                ps_s2 = pb1.tile([E, 1], F32, tag="s2")
                nc.tensor.matmul(ps_s2[:], aggT[:], v2[:], start=True, stop=True)
                nc.scalar.copy(s2p[:], ps_s2[:])

            s2_in = dp.tile([E, 1], F32, tag="s2_in")
            s2_out = dp.tile([E, 1], F32, tag="s2_out")
            nc.sync.dma_start(s2_in[:], s2p[:])
            nc.gpsimd.collective_compute(
                "AllReduce", ALU.add,
                replica_groups=[list(range(NC))],
                ins=[s2_in.opt()], outs=[s2_out.opt()],
            )
            s2f = pbs.tile([E, 1], F32)
            nc.sync.dma_start(s2f[:], s2_out[:])

            attd = pbs.tile([128, NCH * E], F32)
            S_sb = pbs.tile([E, H], F32)
            attdT = pbs.tile([E, NL], F32)
            with tc.tile_pool(name="psB2", bufs=1, space="PSUM") as pb2:
                # s2 row + broadcast to 128 partitions (c12 folded in)
                ps_s2r = pb2.tile([1, E], F32, tag="s2r")
                nc.tensor.transpose(ps_s2r[:], s2f[:], identf[0:E, 0:E])
                s2r = pbs.tile([1, E], F32)
                nc.scalar.activation(
                    s2r[:], ps_s2r[:], AF.Identity, bias=bias[0:1, 5:6]
                )
                ps_s2b = pb2.tile([128, E], F32, tag="s2b")
                nc.tensor.matmul(ps_s2b[:], ones1[:], s2r[:], start=True, stop=True)
                s2b = pbs.tile([128, E], F32)
                nc.scalar.copy(s2b[:], ps_s2b[:])

                ps_S = pb2.tile([E, H], F32, tag="S")
                for c in range(NCH):
                    cs = slice(128 * c, 128 * (c + 1))
                    es = slice(E * c, E * (c + 1))
                    # scores for chunk c: prelu(s2b + s1[:,c]); scores are
                    # O(1000s) (agg sums 8192 nodes) so exp needs max-sub
                    lr = wk.tile([128, E], F32, tag="lr")
                    nc.scalar.activation(
                        lr[:], s2b[:], AF.Prelu, bias=s1[:, c : c + 1], alpha=SLOPE
                    )
                    nmx = wk.tile([128, 1], F32, tag="nmx")
                    nc.vector.tensor_reduce(nmx[:], lr[:], AX.X, ALU.max, negate=True)
                    se = wk.tile([128, 1], F32, tag="se")
                    ex = wk.tile([128, E], F32, tag="ex")
                    nc.scalar.activation(ex[:], lr[:], AF.Exp, bias=nmx[:], accum_out=se[:])
                    rs = wk.tile([128, 1], F32, tag="rs")
                    nc.vector.reciprocal(rs[:], se[:])
                    nc.vector.tensor_scalar(
                        attd[:, es], ex[:], rs[:], invdv[:, c : c + 1],
                        ALU.mult, ALU.mult,
                    )
                    nc.tensor.matmul(
                        ps_S[:], attd[:, es], hid_nm[:, cs],
                        start=(c == 0), stop=(c == NCH - 1),
                    )
                nc.scalar.copy(S_sb[:], ps_S[:])

                for c in range(NCH):
                    ps_t2 = pb2.tile([E, 128], F32, tag="t2", bufs=2)
                    nc.tensor.transpose(
                        ps_t2[:], attd[:, E * c : E * (c + 1)], identf[:]
                    )
                    nc.scalar.copy(attdT[:, 128 * c : 128 * (c + 1)], ps_t2[:])

            if DEBUG:
                nc.sync.dma_start(dbg_s1_d[:], s1[:])
                nc.sync.dma_start(dbg_s2_d[:], s2f[:])
                nc.sync.dma_start(dbg_att_d[:], attd[:])
                nc.sync.dma_start(dbg_S_d[:], S_sb[:])

            S_in = dp.tile([E, H], F32, tag="S_in")
            S_out = dp.tile([E, H], F32, tag="S_out")
            nc.sync.dma_start(S_in[:], S_sb[:])
            nc.gpsimd.collective_compute(
                "AllReduce", ALU.add,
                replica_groups=[list(range(NC))],
                ins=[S_in.opt()], outs=[S_out.opt()],
            )
            SF = pbs.tile([E, H], F32)
            nc.sync.dma_start(SF[:], S_out[:])
            Sd = pbs.tile([E, H], F32)
            nc.vector.tensor_scalar_mul(Sd[:], SF[:], invde[:])

            h2 = pbs.tile([H, NL], F32)
            fc = pbs.tile([H, NL], F32)
            y_sb = pbs.tile([1, NL], F32)
            with tc.tile_pool(name="psB3", bufs=1, space="PSUM") as pb3:
                ps_g2 = pb3.tile([H, NL], F32, tag="g2")
                for c in (0, 512):
                    nc.tensor.matmul(
                        ps_g2[:, c : c + 512], Sd[:], attdT[:, c : c + 512],
                        start=True, stop=True,
                    )
                nc.vector.tensor_add(h2[:], ps_g2[:], h1[:])
                ps_fc = pb3.tile([H, NL], F32, tag="fc")
                for c in (0, 512):
                    nc.tensor.matmul(
                        ps_fc[:, c : c + 512], wfcT[:], h2[:, c : c + 512],
                        start=True, stop=True,
                    )
                nc.scalar.activation(
                    fc[:], ps_fc[:], AF.Prelu, bias=bias[:, 6:7], alpha=SLOPE
                )
                ps_out = pb3.tile([1, NL], F32, tag="out")
                for c in (0, 512):
                    nc.tensor.matmul(
                        ps_out[:, c : c + 512], wout[:], fc[:, c : c + 512],
                        start=True, stop=True,
                    )
                nc.scalar.activation(
                    y_sb[:], ps_out[:], AF.Identity, bias=bias[0:1, 7:8]
                )
            nc.sync.dma_start(y_d[:], y_sb[:])

    nc.finalize()
    return nc


def _prep_inputs(x, GH, Wih0, Whh0, bih0, bhh0, Wih1, Whh1, bih1, bhh1,
                 Wt, bt, a, Wfc, bfc, Wout, bout):
    bf = ml_dtypes.bfloat16
    f32 = np.float32

    a1, a2 = a[:H, 0].astype(f32), a[H:, 0].astype(f32)
    v1 = (Wt.T.astype(f32) @ a1).reshape(H, 1)
    v2 = (Wt.T.astype(f32) @ a2).reshape(H, 1)
    c12 = float(bt.astype(f32) @ a1 + bt.astype(f32) @ a2)

    de = GH.astype(f32).sum(axis=0)
    dv = GH.astype(f32).sum(axis=1) / 2.0
    inv_de = np.where(de != 0, 1.0 / np.where(de != 0, de, 1.0), 0.0).astype(f32)
    inv_dv = np.where(dv != 0, 1.0 / np.where(dv != 0, dv, 1.0), 0.0).astype(f32)

    def neg_z(wT):
        # wT: [*, 3H] transposed weights; negate the z block -> w-gate
        w = wT.copy()
        w[:, H : 2 * H] *= -1.0
        return w

    wihT0_aug = np.zeros((7, 3 * H), f32)
    wihT0_aug[:6] = Wih0.T
    wihT0_aug[6, 0:H] = bih0[0:H] + bhh0[0:H]
    wihT0_aug[6, H : 2 * H] = bih0[H : 2 * H] + bhh0[H : 2 * H]
    wihT0_aug[6, 2 * H :] = bih0[2 * H :]
    wihT0_aug = neg_z(wihT0_aug)

    bias = np.zeros((H, 8), f32)
    bias[:, 0] = bih1[0:H] + bhh1[0:H]
    bias[:, 1] = -(bih1[H : 2 * H] + bhh1[H : 2 * H])
    bias[:, 2] = bhh0[2 * H :]
    bias[:, 3] = bhh1[2 * H :]
    bias[:, 4] = bih1[2 * H :]
    bias[:, 5] = c12
    bias[:, 6] = bfc
    bias[:, 7] = float(bout[0])

    shared = {
        "whhT0": neg_z(np.ascontiguousarray(Whh0.T).astype(f32)).astype(bf),
        "wihT0": wihT0_aug.astype(bf),
        "whhT1": neg_z(np.ascontiguousarray(Whh1.T).astype(f32)).astype(bf),
        "wihT1": neg_z(np.ascontiguousarray(Wih1.T).astype(f32)).astype(bf),
        "bias": bias,
        "v1": v1.astype(bf),
        "v2": v2,
        "wfcT": np.ascontiguousarray(Wfc.T).astype(f32),
        "wout": np.ascontiguousarray(Wout[0].reshape(H, 1)).astype(f32),
        "identd": np.eye(H, dtype=f32).astype(bf),
        "identf": np.eye(H, dtype=f32),
        "ones1": np.ones((1, H), f32),
        "invde": inv_de.reshape(E, 1),
    }

    in_maps = []
    for ci in range(NC):
        n0 = ci * NL
        xc = x[n0 : n0 + NL, :T, :].astype(f32)  # [NL, T, DF]
        xa = np.ones((7, T, NL), f32)
        xa[:6] = xc.transpose(2, 1, 0)
        ghc = GH[n0 : n0 + NL].astype(f32)  # [NL, E]
        gh_nm = ghc.reshape(NCH, 128, E).transpose(1, 0, 2).reshape(128, NCH * E)
        invdv_nm = inv_dv[n0 : n0 + NL].reshape(NCH, 128).T.copy()
        m = dict(shared)
        m["x"] = xa.reshape(7, T * NL).astype(bf)
        m["gh"] = np.ascontiguousarray(gh_nm)
        m["invdv"] = np.ascontiguousarray(invdv_nm)
        in_maps.append(m)
    return in_maps


def kernel(**inputs):
    if "nc" not in _CACHE:
        _CACHE["nc"] = _build_program()
    nc = _CACHE["nc"]
    in_maps = _prep_inputs(**inputs)
    res = run_bass_kernel_spmd(nc, in_maps, list(range(NC)))
    out = np.concatenate([res.results[i]["y"][0] for i in range(NC)])
    return out.astype(np.float32)


def _install_profile_shim():
    """Recreate the antenv.axon_hooks NTFF profile hook missing from this image."""
    import types
    import ctypes
    import contextlib

    if "antenv.axon_hooks" in sys.modules:
        return
    so_path = "/opt/axon/libaxon_pjrt.so"
    lib = ctypes.CDLL(so_path)
    lib.axon_start_nrt_profile.argtypes = [
        ctypes.POINTER(ctypes.c_int64), ctypes.c_size_t,
    ]
    lib.axon_start_nrt_profile.restype = ctypes.c_int64
    lib.axon_stop_nrt_profile.argtypes = [ctypes.c_char_p]
    lib.axon_stop_nrt_profile.restype = ctypes.c_int64

    @contextlib.contextmanager
    def _hook(output_dir, device_ids):
        import jax

        jax.devices()
        if device_ids:
            ids = (ctypes.c_int64 * len(device_ids))(*device_ids)
            rc = lib.axon_start_nrt_profile(ids, len(device_ids))
        else:
            rc = lib.axon_start_nrt_profile(None, 0)
        if rc != 0:
            raise RuntimeError(f"axon_start_nrt_profile rc={rc}")
        try:
            yield
        finally:
            n = lib.axon_stop_nrt_profile(str(output_dir).encode())
            print(f"profile: {n} file(s) written to {output_dir}")

    mod = types.ModuleType("antenv.axon_hooks")
    mod.get_axon_ntff_profile_hook = lambda: _hook
    mod.set_axon_ntff_profile_hook = lambda h: None
    sys.modules["antenv.axon_hooks"] = mod
    import antenv

    antenv.axon_hooks = mod

    import concourse.bass_utils as bu

    bu.upload_artifacts = lambda tmpdir: f"local://{tmpdir}"


def run_traced(inputs, tmpdir=None):
    """test.py helper: run with NTFF tracing, return (output, BassKernelResults)."""
    _install_profile_shim()
    if "nc" not in _CACHE:
        _CACHE["nc"] = _build_program()
    nc = _CACHE["nc"]
    in_maps = _prep_inputs(**inputs)
    res = run_bass_kernel_spmd(
        nc, in_maps, list(range(NC)), trace=True, tmpdir=tmpdir
    )
    out = np.concatenate([res.results[i]["y"][0] for i in range(NC)])
    return out.astype(np.float32), res
